# revision 66
# baseline (speedup 1.0000x reference)
"""2-layer GCN (DBPnet GCN head) on 8 Trainium2 NeuronCores.

Algorithm (matches the jax reference):
    x0 = relu(x)
    x1 = relu(gcn_conv(x0, W1, b1))
    x2 = gcn_conv(x1, W2, b2)
    y  = softmax(x2, axis=-1)
with gcn_conv(x) = D^-1/2 (A + I) D^-1/2 (x @ W) + b  (in-degree over dst + 1).

Sharding: nodes row-partitioned over 8 cores (6250 each); edges partitioned
by destination core.  Per layer each core computes hs = dinv * (x_shard @ W),
all-gathers hs into a full table (split into two src-half tables A/B so the
collectives overlap edge processing and dma_gather int16 indices stay in
range), gathers hs[src] rows for its dst-sorted edges with batched indirect
DMAs, and segment-sums on the tensor engine using one-hot selection matrices
S (S[e, j] = dst_slot[e] == j) accumulated in PSUM:
    psum_w = sum_tiles S_tile^T @ gathered_tile
    out_w  = act( dinv_w * (psum_w + hs_w) + b )

Edge tiles straddle dst-window boundaries (no per-window padding): a tile
whose 128 edges span 2-3 windows issues one matmul per window ("plane"),
with the one-hot built from slot values offset by 128*plane.  The dst_slot
stream encodes (window - tile_base_window)*128 + slot; plane-k selection
matrices come from comparing against iota+128k.

The one-hot decomposition dinv[src]*dinv[dst] = (dinv applied pre-allgather)
* (dinv applied post-aggregation) makes the per-edge norm free.
"""

import sys

import numpy as np

sys.path.insert(0, "/opt/trn_rl_repo")

import ml_dtypes  # noqa: E402
from concourse import bass, mybir  # noqa: E402
import concourse.bacc as bacc  # noqa: E402
import concourse.tile as tile  # noqa: E402
from concourse.bass_utils import run_bass_kernel_spmd  # noqa: E402

F32 = mybir.dt.float32
BF16 = mybir.dt.bfloat16
I16 = mybir.dt.int16

C = 8            # cores
P = 128          # partitions / edge-tile size / window size
TG = 8           # edge tiles per gather DMA (dma_gather num_idxs = TG*128;
                 # single_packet packs 64 descriptors per engine = 1024 idxs
                 # max; single_packet=False allows more but slows the DMA
                 # engines to a packet per descriptor - net loss)
TB = 16          # edge tiles per S-build op (plane-0 and plane-1 batches)
PAD_SLOT = 300.0  # dst_slot value for padding edges (no iota match;
                  # must be bf16-exact and > 255)
QW = [0, 6, 28, 49]  # pass boundaries, in dst-window units: each pass
                     # gathers from the table slice holding local shard
                     # rows [QW[p]*128, QW[p+1]*128).  The small first slice
                     # makes its all-gather fire early (short lead-in); all
                     # slices keep 8*rows < 2^15 (int16 gather indices).


# ---------------------------------------------------------------- host prep

def _schedule(src, dst, N):
    """Straddle-tile schedule, shared program structure for all cores.

    Edges are dst-sorted per core, split into passes by source quarter
    (pass p covers edges whose source's local shard offset falls in
    quarter p, so the gather table for pass p is the all-gather of that
    quarter), and packed densely into 128-edge tiles with no window
    alignment.  The compile-time structure (tile -> base window w0, plane
    count; window -> first/last tile) is the union over all cores, so one
    SPMD program serves all.

    Returns (meta, per_core) where
      meta = [ (T_h, w0_t[T], npl_t[T], first_t[W], last_t[W]) per pass ]
      per_core[c] = (idx_wrapped [P, T_tot*8] int16, slots [P, 2*T_tot] bf16)
    """
    NS = N // C
    W = (NS + P - 1) // P
    NP = len(QW)  # pass 0: own-shard edges (local table, no collective);
                  # passes 1..: remote edges by source quarter
    qlo = [min(q * P, NS) for q in QW]           # quarter row bounds

    order = np.argsort(dst, kind="stable")
    s_dst = dst[order]
    s_src = src[order]
    core_bounds = np.searchsorted(s_dst, np.arange(C + 1) * NS)

    streams = [[None] * C for _ in range(NP)]  # [h][c] -> (w, slot, tabi)
    for c in range(C):
        lo, hi = core_bounds[c], core_bounds[c + 1]
        d_loc = (s_dst[lo:hi] - c * NS).astype(np.int64)
        sc = s_src[lo:hi].astype(np.int64)
        sh, off = sc // NS, sc % NS
        h_e = np.where(sh == c, 0,
                       1 + np.searchsorted(qlo[1:-1], off, side="right"))
        w_e = d_loc >> 7
        slot = d_loc & 127
        for h in range(NP):
            m = h_e == h
            if h == 0:
                tabi = off[m]
            else:
                qr = qlo[h] - qlo[h - 1]
                tabi = sh[m] * qr + (off[m] - qlo[h - 1])
            streams[h][c] = (w_e[m], slot[m], tabi)

    meta = []
    sl_all = []
    si_all = []
    for h in range(NP):
        E_h = max(len(streams[h][c][0]) for c in range(C))
        T_h = (E_h + P - 1) // P
        L = T_h * P
        wE = np.full((C, L), -1, np.int64)
        slE = np.zeros((C, L), np.int64)
        siE = np.zeros((C, L), np.int16)
        for c in range(C):
            w_e, slot, tabi = streams[h][c]
            n = len(w_e)
            wE[c, :n] = w_e
            slE[c, :n] = slot
            siE[c, :n] = tabi.astype(np.int16)
        wT = wE.reshape(C, T_h, P)
        wmask = np.ma.masked_equal(wT, -1)
        w0_t = np.asarray(wmask.min(axis=(0, 2)))       # [T]
        wmax_t = np.asarray(wmask.max(axis=(0, 2)))
        npl_t = wmax_t - w0_t + 1
        assert npl_t.max() <= 3, f"tile spans {npl_t.max()} windows"
        # window -> first/last tile with that window in span
        first_t = np.full(W, -1, np.int64)
        last_t = np.full(W, -1, np.int64)
        for t in range(T_h):
            for w in range(int(w0_t[t]), int(wmax_t[t]) + 1):
                if first_t[w] < 0:
                    first_t[w] = t
                last_t[w] = t
        assert (first_t >= 0).all()
        # two slot streams (bf16 integers are exact only up to 256):
        #   stream0: plane-0 edges -> slot (0..127), else PAD
        #   stream1: plane-1 edges -> slot, plane-2 -> 128+slot, else PAD
        plane = wE - w0_t.repeat(P)[None, :]
        v0 = np.where((wE >= 0) & (plane == 0), slE, PAD_SLOT)
        v1 = np.where((wE >= 0) & (plane == 1), slE,
                      np.where((wE >= 0) & (plane == 2), P + slE, PAD_SLOT))
        meta.append((T_h, w0_t.tolist(), npl_t.tolist(),
                     first_t.tolist(), last_t.tolist()))
        sl_all.append((v0.astype(np.float32), v1.astype(np.float32)))
        si_all.append(siE)

    per_core = []
    for c in range(C):
        si = np.concatenate([si_all[h][c] for h in range(NP)])
        sl0 = np.concatenate([sl_all[h][0][c] for h in range(NP)])
        sl1 = np.concatenate([sl_all[h][1][c] for h in range(NP)])
        T_tot = len(si) // P
        siw = np.ascontiguousarray(
            np.tile(si.reshape(T_tot * 8, 16).T, (8, 1)))
        sl = np.ascontiguousarray(np.concatenate(
            [sl0.reshape(T_tot, P).T, sl1.reshape(T_tot, P).T],
            axis=1).astype(ml_dtypes.bfloat16))
        per_core.append((siw, sl))
    return meta, per_core


# ------------------------------------------------------------- device build

def build_program(nc, N, H, F1, F2, meta, cc=True):
    """Emit the SPMD program. All cores run identical code; per-core data
    comes in through the input tensors."""
    NS = N // C
    W = (NS + P - 1) // P
    NSP = W * P
    NPASS = len(QW)      # pass 0: own-shard (local table); 1..: quarters
    NQ = len(QW) - 1
    qlo = [min(q * P, NS) for q in QW]   # quarter row bounds (local)
    qrows = [qlo[q + 1] - qlo[q] for q in range(NQ)]
    T_tot = sum(m[0] for m in meta)

    d_xT = nc.dram_tensor("xT", [H, NSP], BF16, kind="ExternalInput")
    d_W1 = nc.dram_tensor("W1", [H, F1], BF16, kind="ExternalInput")
    d_W2 = nc.dram_tensor("W2", [F1, F2], BF16, kind="ExternalInput")
    d_b1 = nc.dram_tensor("b1r", [P, F1], F32, kind="ExternalInput")
    d_b2 = nc.dram_tensor("b2r", [P, F2], F32, kind="ExternalInput")
    d_dinv = nc.dram_tensor("dinv", [P, W], F32, kind="ExternalInput")
    d_iota = nc.dram_tensor("iota", [P, 2 * P], BF16, kind="ExternalInput")
    d_ident = nc.dram_tensor("ident", [P, P], BF16, kind="ExternalInput")
    d_si = nc.dram_tensor("srcidx", [P, T_tot * 8], I16, kind="ExternalInput")
    d_sl = nc.dram_tensor("dstslot", [P, 2 * T_tot], BF16,
                          kind="ExternalInput")
    d_y = nc.dram_tensor("y", [NS, F2], F32, kind="ExternalOutput")

    with tile.TileContext(nc) as tc:
        with (
            tc.tile_pool(name="const", bufs=1) as const_pool,
            tc.tile_pool(name="persist", bufs=1) as persist,
            tc.tile_pool(name="gath", bufs=4) as gath_pool,
            tc.tile_pool(name="sbuild", bufs=3) as s_pool,
            tc.tile_pool(name="sdual", bufs=6) as sd_pool,
            tc.tile_pool(name="winbuf", bufs=3) as win_pool,
            tc.tile_pool(name="small", bufs=6) as small_pool,
            tc.tile_pool(name="agg", bufs=4, space="PSUM") as psum_agg,
            tc.tile_pool(name="dense", bufs=2, space="PSUM") as psum_dense,
            tc.tile_pool(name="tpose", bufs=2, space="PSUM") as psum_t,
            tc.tile_pool(name="dram", bufs=1, space="DRAM") as dram,
        ):
            # ---- constants / persistent state -----------------------------
            # dense-phase criticals first: the sync engine issues in order
            sb_xT = persist.tile([H, NSP], BF16, tag="xT")
            Q1 = QW[1] * P
            nc.sync.dma_start(out=sb_xT[:, :Q1], in_=d_xT[:, :Q1])
            sb_W1 = const_pool.tile([H, F1], BF16, tag="w1")
            nc.sync.dma_start(out=sb_W1[:], in_=d_W1[:])
            sb_dinv = const_pool.tile([P, W], F32, tag="dinv")
            nc.sync.dma_start(out=sb_dinv[:], in_=d_dinv[:])
            nc.sync.dma_start(out=sb_xT[:, Q1:], in_=d_xT[:, Q1:])
            sb_iota = const_pool.tile([P, 2, P], BF16, tag="iota")
            nc.scalar.dma_start(
                out=sb_iota[:], in_=d_iota[:].rearrange("p (k n) -> p k n", k=2))
            sb_W2 = const_pool.tile([F1, F2], BF16, tag="w2")
            nc.scalar.dma_start(out=sb_W2[:], in_=d_W2[:])
            sb_b1 = const_pool.tile([P, F1], F32, tag="b1")
            nc.scalar.dma_start(out=sb_b1[:], in_=d_b1[:])
            sb_b2 = const_pool.tile([P, F2], F32, tag="b2")
            nc.scalar.dma_start(out=sb_b2[:], in_=d_b2[:])
            sb_ident = const_pool.tile([P, P], BF16, tag="ident")
            nc.scalar.dma_start(out=sb_ident[:], in_=d_ident[:])
            # big edge tables last: they are not needed until the first
            # gather (~90us in) and would steal DMA bandwidth from xT
            sb_si = const_pool.tile([P, T_tot * 8], I16, tag="srcidx")
            nc.scalar.dma_start(out=sb_si[:], in_=d_si[:])
            sb_sl = const_pool.tile([P, 2 * T_tot], BF16, tag="dstslot")
            nc.scalar.dma_start(out=sb_sl[:], in_=d_sl[:])

            sb_hs1 = persist.tile([P, W, F1], BF16, tag="hs1")
            sb_hs2 = persist.tile([P, W, F2], F32, tag="hs2")
            sb_acc1 = persist.tile([P, W, F1], F32, tag="acc1")
            sb_acc2 = persist.tile([P, W, F2], F32, tag="acc2")

            # DRAM bounce + gather tables (own shard + per source quarter)
            hs1_own = dram.tile([NS, F1], BF16, name="hs1_own")
            hs2_own = dram.tile([NS, F2], F32, name="hs2_own")
            hs1_loc, hs1_full, hs2_loc, hs2_full = [], [], [], []
            for q in range(NQ):
                t1l = dram.tile([qrows[q], F1], BF16, name=f"hs1_loc{q}")
                t1f = dram.tile([C * qrows[q], F1], BF16,
                                name=f"hs1_full{q}", addr_space="Shared")
                t2l = dram.tile([qrows[q], F2], F32, name=f"hs2_loc{q}")
                t2f = dram.tile([C * qrows[q], F2], F32,
                                name=f"hs2_full{q}", addr_space="Shared")
                hs1_loc.append(t1l)
                hs1_full.append(t1f)
                hs2_loc.append(t2l)
                hs2_full.append(t2f)

            def allgather(src_t, dst_t):
                if cc:
                    nc.gpsimd.collective_compute(
                        "AllGather", mybir.AluOpType.bypass,
                        replica_groups=[list(range(C))],
                        ins=[src_t[:].opt()], outs=[dst_t[:].opt()])
                else:
                    rows = src_t.shape[0]
                    nc.sync.dma_start(out=dst_t[:rows, :], in_=src_t[:])



            def flush_quarter(loc_list, sb, q, F):
                """Batched write of one source-quarter of the window table:
                the full-window span as one partition-major DMA, plus the
                ragged last window (w = W-1) separately."""
                w0, w1 = QW[q], QW[q + 1]
                full_w = w1 - w0 if q < NQ - 1 else w1 - w0 - 1
                nc.scalar.dma_start(
                    out=loc_list[q][:full_w * P, :].rearrange(
                        "(w p) f -> p w f", p=P),
                    in_=sb[:, w0:w0 + full_w, :])
                if q == NQ - 1:
                    rows = NS - (W - 1) * P
                    nc.scalar.dma_start(
                        out=loc_list[q][full_w * P:, :],
                        in_=sb[:rows, W - 1, :])

            def flush_own(own_t, sb):
                """Whole-shard local table write (pass 0 gathers from it)."""
                nc.scalar.dma_start(
                    out=own_t[:(W - 1) * P, :].rearrange(
                        "(w p) f -> p w f", p=P),
                    in_=sb[:, :W - 1, :])
                rows = NS - (W - 1) * P
                nc.scalar.dma_start(out=own_t[(W - 1) * P:, :],
                                    in_=sb[:rows, W - 1, :])

            # ---- phase 1: x0 = relu(x); hs1 = dinv * (x0 @ W1) ------------
            # psum region of 4 windows per bank; one batched dinv-mul per 4
            nc.vector.tensor_scalar_max(sb_xT[:, :Q1], sb_xT[:, :Q1], 0.0)
            nc.vector.tensor_scalar_max(sb_xT[:, Q1:], sb_xT[:, Q1:], 0.0)
            for q in range(NQ):
                for w0 in range(QW[q], QW[q + 1], 4):
                    n = min(4, QW[q + 1] - w0)
                    ph = psum_dense.tile([P, 4, F1], F32, tag="dense")
                    for k in range(n):
                        w = w0 + k
                        nc.tensor.matmul(ph[:, k, :],
                                         lhsT=sb_xT[:, w * P:(w + 1) * P],
                                         rhs=sb_W1[:], start=True, stop=True)
                    nc.vector.tensor_tensor(
                        out=sb_hs1[:, w0:w0 + n, :], in0=ph[:, :n, :],
                        in1=sb_dinv[:, w0:w0 + n].to_broadcast([P, n, F1]),
                        op=mybir.AluOpType.mult)
                flush_quarter(hs1_loc, sb_hs1, q, F1)
                allgather(hs1_loc[q], hs1_full[q])
            flush_own(hs1_own, sb_hs1)

            # ---- edge aggregation (both layers) ---------------------------
            def edge_layer(tables, F, dt, acc_sb, out_cb, ag_hook=None):
                """Two passes (one per src half); pass 0 parks the partial
                window sums in acc_sb, pass 1 finishes and calls out_cb.
                ag_hook(w) is called after window w is finalized in pass 1."""
                t_base = 0
                pa_w = {}
                for h in range(NPASS):
                    T_h, w0_t, npl_t, first_t, last_t = meta[h]
                    tab = tables[h]
                    # S batches built just-in-time inside the tile loop so the
                    # in-order vector queue interleaves builds with epilogues:
                    # plane-0 from stream0, plane-1 from stream1 (zero
                    # matrices for tiles without a plane-1 — harmless)
                    sts = {}
                    sts1 = {}
                    gts = {}
                    for t in range(T_h):
                        if t % TB == 0:
                            n = min(TB, T_h - t)
                            s = s_pool.tile([P, TB, P], dt, tag="sbuild")
                            nc.vector.tensor_tensor(
                                out=s[:, :n, :],
                                in0=sb_sl[:, t_base + t:t_base + t + n
                                          ].to_broadcast([P, n, P]),
                                in1=sb_iota[:, 0:1, :].to_broadcast([P, n, P]),
                                op=mybir.AluOpType.is_equal)
                            sts[t // TB] = s
                            if any(npl_t[tt] > 1 for tt in range(t, t + n)):
                                s1 = s_pool.tile([P, TB, P], dt, tag="sbuild1")
                                nc.vector.tensor_tensor(
                                    out=s1[:, :n, :],
                                    in0=sb_sl[:, T_tot + t_base + t:
                                              T_tot + t_base + t + n
                                              ].to_broadcast([P, n, P]),
                                    in1=sb_iota[:, 0:1, :].to_broadcast(
                                        [P, n, P]),
                                    op=mybir.AluOpType.is_equal)
                                sts1[t // TB] = s1
                        if t % TG == 0:
                            g = gath_pool.tile([P, TG, F], dt, tag="gath")
                            n = min(TG, T_h - t)
                            nc.gpsimd.dma_gather(
                                g[:, :n, :], tab[:],
                                sb_si[:, (t_base + t) * 8:(t_base + t + n) * 8],
                                n * P, n * P, F)
                            gts[t // TG] = g
                        rhs = gts[t // TG][:, t % TG, :]
                        for k in range(npl_t[t]):
                            w = w0_t[t] + k
                            if first_t[w] == t:
                                pa = psum_agg.tile([P, F1], F32, tag="agg",
                                                   name=f"pa{h}_{w}")
                                pa_w[w] = pa
                            if k == 0:
                                s_plane = sts[t // TB][:, t % TB, :]
                            elif k == 1:
                                s_plane = sts1[t // TB][:, t % TB, :]
                            else:
                                col = T_tot + t_base + t
                                sd = sd_pool.tile([P, 1, P], dt, tag="sd")
                                nc.vector.tensor_tensor(
                                    out=sd[:],
                                    in0=sb_sl[:, col:col + 1
                                              ].to_broadcast([P, 1, P]),
                                    in1=sb_iota[:, 1:2, :],
                                    op=mybir.AluOpType.is_equal)
                                s_plane = sd[:, 0, :]
                            nc.tensor.matmul(
                                pa_w[w][:, :F], lhsT=s_plane, rhs=rhs,
                                start=(first_t[w] == t),
                                stop=(last_t[w] == t))
                        for k in range(npl_t[t]):
                            w = w0_t[t] + k
                            if last_t[w] != t:
                                continue
                            pa = pa_w.pop(w)
                            rows = min(P, NS - w * P)
                            if h == 0:
                                nc.scalar.activation(
                                    acc_sb[:, w, :], pa[:, :F],
                                    mybir.ActivationFunctionType.Copy)
                            elif h < NPASS - 1:
                                nc.vector.tensor_tensor(
                                    out=acc_sb[:, w, :], in0=pa[:, :F],
                                    in1=acc_sb[:, w, :],
                                    op=mybir.AluOpType.add)
                            else:
                                out_cb(w, rows, pa[:, :F])
                                if ag_hook is not None:
                                    ag_hook(w)
                    t_base += T_h

            # ---- layer-1 epilogue: relu, transpose, dense L2 --------------
            def l1_out(w, rows, pa):
                tmp = win_pool.tile([P, F1], F32, tag="tmp")
                nc.vector.tensor_tensor(out=tmp[:], in0=pa,
                                        in1=sb_acc1[:, w, :],
                                        op=mybir.AluOpType.add)
                nc.vector.tensor_tensor(out=tmp[:], in0=tmp[:],
                                        in1=sb_hs1[:, w, :],
                                        op=mybir.AluOpType.add)
                nc.vector.tensor_scalar_mul(tmp[:], tmp[:], sb_dinv[:, w:w + 1])
                nc.vector.tensor_tensor(out=tmp[:], in0=tmp[:], in1=sb_b1[:],
                                        op=mybir.AluOpType.add)
                x1 = win_pool.tile([P, F1], BF16, tag="x1")
                nc.scalar.activation(x1[:], tmp[:],
                                     mybir.ActivationFunctionType.Relu)
                # transpose x1 -> lhsT for the layer-2 dense matmul
                pt = psum_t.tile([P, P], BF16, tag="tpose")
                nc.tensor.transpose(pt[:], x1[:], sb_ident[:])
                x1T = win_pool.tile([P, P], BF16, tag="x1T")
                nc.vector.tensor_copy(x1T[:], pt[:])
                ph2 = psum_dense.tile([P, F1], F32, tag="dense")
                nc.tensor.matmul(ph2[:, :F2], lhsT=x1T[:], rhs=sb_W2[:],
                                 start=True, stop=True)
                nc.vector.tensor_scalar_mul(
                    sb_hs2[:, w, :], ph2[:, :F2], sb_dinv[:, w:w + 1])

            qend = {QW[q + 1] - 1: q for q in range(NQ)}

            def l1_ag(w):
                if w in qend:
                    q = qend[w]
                    flush_quarter(hs2_loc, sb_hs2, q, F2)
                    allgather(hs2_loc[q], hs2_full[q])
                    if w == W - 1:
                        flush_own(hs2_own, sb_hs2)

            edge_layer([hs1_own] + hs1_full, F1, BF16, sb_acc1, l1_out,
                       ag_hook=l1_ag)

            # ---- layer-2 edges + softmax ----------------------------------
            def l2_out(w, rows, pa):
                tmp = win_pool.tile([P, F2], F32, tag="tmp2")
                nc.vector.tensor_tensor(out=tmp[:], in0=pa,
                                        in1=sb_acc2[:, w, :],
                                        op=mybir.AluOpType.add)
                nc.vector.tensor_tensor(out=tmp[:], in0=tmp[:],
                                        in1=sb_hs2[:, w, :],
                                        op=mybir.AluOpType.add)
                nc.vector.tensor_scalar_mul(tmp[:], tmp[:], sb_dinv[:, w:w + 1])
                nc.vector.tensor_tensor(out=tmp[:], in0=tmp[:], in1=sb_b2[:],
                                        op=mybir.AluOpType.add)
                nmax = small_pool.tile([P, 1], F32, tag="nmax")
                nc.vector.tensor_reduce(nmax[:], tmp[:],
                                        axis=mybir.AxisListType.X,
                                        op=mybir.AluOpType.max, negate=True)
                ex = win_pool.tile([P, F2], F32, tag="ex")
                ssum = small_pool.tile([P, 1], F32, tag="ssum")
                nc.scalar.activation(ex[:], tmp[:],
                                     mybir.ActivationFunctionType.Exp,
                                     bias=nmax[:], accum_out=ssum[:])
                rsum = small_pool.tile([P, 1], F32, tag="rsum")
                nc.vector.reciprocal(rsum[:], ssum[:])
                yw = win_pool.tile([P, F2], F32, tag="yw")
                nc.vector.tensor_scalar_mul(yw[:], ex[:], rsum[:])
                nc.scalar.dma_start(out=d_y[w * P:w * P + rows, :],
                                    in_=yw[:rows, :])

            edge_layer([hs2_own] + hs2_full, F2, F32, sb_acc2, l2_out)

    return {"out_name": "y"}


# ---------------------------------------------------------------- frontend

_CACHE = {}


def _meta_key(meta):
    return tuple((T, tuple(w0), tuple(npl), tuple(ft), tuple(lt))
                 for T, w0, npl, ft, lt in meta)


def _build_and_compile(N, H, F1, F2, meta):
    nc = bacc.Bacc("TRN2", target_bir_lowering=False, debug=False,
                   enable_asserts=False, num_devices=C)
    build_program(nc, N, H, F1, F2, meta)
    nc.compile()
    return nc


def prepare_inputs(x, edge_index, W1, b1, W2, b2):
    N, H = x.shape
    F1 = W1.shape[1]
    F2 = W2.shape[1]
    NS = N // C
    W = (NS + P - 1) // P
    NSP = W * P

    src = np.asarray(edge_index[0], dtype=np.int64)
    dst = np.asarray(edge_index[1], dtype=np.int64)
    deg = np.bincount(dst, minlength=N).astype(np.float32) + 1.0
    dinv = (1.0 / np.sqrt(deg)).astype(np.float32)

    meta, per_core = _schedule(src, dst, N)

    iota = np.ascontiguousarray(np.tile(
        np.arange(2 * P, dtype=np.float32).reshape(1, -1),
        (P, 1)).astype(ml_dtypes.bfloat16))
    ident = np.eye(P, dtype=np.float32).astype(ml_dtypes.bfloat16)
    b1r = np.ascontiguousarray(np.tile(np.asarray(b1, np.float32), (P, 1)))
    b2r = np.ascontiguousarray(np.tile(np.asarray(b2, np.float32), (P, 1)))
    W1f = np.ascontiguousarray(
        np.asarray(W1, np.float32).astype(ml_dtypes.bfloat16))
    W2f = np.ascontiguousarray(
        np.asarray(W2, np.float32).astype(ml_dtypes.bfloat16))

    in_maps = []
    for c in range(C):
        xs = np.zeros((NSP, H), np.float32)
        xs[:NS] = np.asarray(x[c * NS:(c + 1) * NS], np.float32)
        xT = np.ascontiguousarray(xs.T.astype(ml_dtypes.bfloat16))
        dv = np.ones(NSP, np.float32)
        dv[:NS] = dinv[c * NS:(c + 1) * NS]
        dv = np.ascontiguousarray(dv.reshape(W, P).T)
        si, sl = per_core[c]
        in_maps.append({
            "xT": xT, "W1": W1f, "W2": W2f, "b1r": b1r, "b2r": b2r,
            "dinv": dv, "iota": iota, "ident": ident,
            "srcidx": si, "dstslot": sl,
        })
    return in_maps, (N, H, F1, F2, meta)


def kernel(x, edge_index, W1, b1, W2, b2, trace=False):
    x = np.asarray(x)
    in_maps, key = prepare_inputs(x, edge_index, W1, b1, W2, b2)
    N, H, F1, F2, meta = key
    ck = (N, H, F1, F2, _meta_key(meta))
    if ck not in _CACHE:
        _CACHE.clear()
        _CACHE[ck] = _build_and_compile(N, H, F1, F2, meta)
    nc = _CACHE[ck]
    res = run_bass_kernel_spmd(nc, in_maps, core_ids=list(range(C)),
                               trace=trace)
    y = np.concatenate([res.results[c]["y"] for c in range(C)], axis=0)
    if trace:
        kernel.last_exec_time_ns = res.exec_time_ns
    return y.astype(np.float32)


kernel.last_exec_time_ns = None


# revision 69
# speedup vs baseline: 1.1057x; 1.1057x over previous
"""2-layer GCN (DBPnet GCN head) on 8 Trainium2 NeuronCores.

Algorithm (matches the jax reference):
    x0 = relu(x)
    x1 = relu(gcn_conv(x0, W1, b1))
    x2 = gcn_conv(x1, W2, b2)
    y  = softmax(x2, axis=-1)
with gcn_conv(x) = D^-1/2 (A + I) D^-1/2 (x @ W) + b  (in-degree over dst + 1).

Sharding: nodes row-partitioned over 8 cores (6250 each); edges partitioned
by destination core.  Per layer each core computes hs = dinv * (x_shard @ W),
all-gathers hs into a full table (split into two src-half tables A/B so the
collectives overlap edge processing and dma_gather int16 indices stay in
range), gathers hs[src] rows for its dst-sorted edges with batched indirect
DMAs, and segment-sums on the tensor engine using one-hot selection matrices
S (S[e, j] = dst_slot[e] == j) accumulated in PSUM:
    psum_w = sum_tiles S_tile^T @ gathered_tile
    out_w  = act( dinv_w * (psum_w + hs_w) + b )

Edge tiles straddle dst-window boundaries (no per-window padding): a tile
whose 128 edges span 2-3 windows issues one matmul per window ("plane"),
with the one-hot built from slot values offset by 128*plane.  The dst_slot
stream encodes (window - tile_base_window)*128 + slot; plane-k selection
matrices come from comparing against iota+128k.

The one-hot decomposition dinv[src]*dinv[dst] = (dinv applied pre-allgather)
* (dinv applied post-aggregation) makes the per-edge norm free.
"""

import sys

import numpy as np

sys.path.insert(0, "/opt/trn_rl_repo")

import ml_dtypes  # noqa: E402
from concourse import bass, mybir  # noqa: E402
import concourse.bacc as bacc  # noqa: E402
import concourse.tile as tile  # noqa: E402
from concourse.bass_utils import run_bass_kernel_spmd  # noqa: E402

F32 = mybir.dt.float32
BF16 = mybir.dt.bfloat16
I16 = mybir.dt.int16

C = 8            # cores
P = 128          # partitions / edge-tile size / window size
TG = 8           # edge tiles per gather DMA (dma_gather num_idxs = TG*128;
                 # single_packet packs 64 descriptors per engine = 1024 idxs
                 # max; single_packet=False allows more but slows the DMA
                 # engines to a packet per descriptor - net loss)
TB = 16          # edge tiles per S-build op (plane-0 and plane-1 batches)
PAD_SLOT = 300.0  # dst_slot value for padding edges (no iota match;
                  # must be bf16-exact and > 255)
QW = [0, 6, 28, 49]  # pass boundaries, in dst-window units: each pass
                     # gathers from the table slice holding local shard
                     # rows [QW[p]*128, QW[p+1]*128).  The small first slice
                     # makes its all-gather fire early (short lead-in); all
                     # slices keep 8*rows < 2^15 (int16 gather indices).


# ---------------------------------------------------------------- host prep

def _schedule(src, dst, N):
    """Straddle-tile schedule, shared program structure for all cores.

    Edges are dst-sorted per core, split into passes by source quarter
    (pass p covers edges whose source's local shard offset falls in
    quarter p, so the gather table for pass p is the all-gather of that
    quarter), and packed densely into 128-edge tiles with no window
    alignment.  The compile-time structure (tile -> base window w0, plane
    count; window -> first/last tile) is the union over all cores, so one
    SPMD program serves all.

    Returns (meta, per_core) where
      meta = [ (T_h, w0_t[T], npl_t[T], first_t[W], last_t[W]) per pass ]
      per_core[c] = (idx_wrapped [P, T_tot*8] int16, slots [P, 2*T_tot] bf16)
    """
    NS = N // C
    W = (NS + P - 1) // P
    NP = len(QW)  # pass 0: own-shard edges (local table, no collective);
                  # passes 1..: remote edges by source quarter
    qlo = [min(q * P, NS) for q in QW]           # quarter row bounds

    order = np.argsort(dst, kind="stable")
    s_dst = dst[order]
    s_src = src[order]
    core_bounds = np.searchsorted(s_dst, np.arange(C + 1) * NS)

    streams = [[None] * C for _ in range(NP)]  # [h][c] -> (w, slot, tabi)
    for c in range(C):
        lo, hi = core_bounds[c], core_bounds[c + 1]
        d_loc = (s_dst[lo:hi] - c * NS).astype(np.int64)
        sc = s_src[lo:hi].astype(np.int64)
        sh, off = sc // NS, sc % NS
        h_e = np.where(sh == c, 0,
                       1 + np.searchsorted(qlo[1:-1], off, side="right"))
        w_e = d_loc >> 7
        slot = d_loc & 127
        for h in range(NP):
            m = h_e == h
            if h == 0:
                tabi = off[m]
            else:
                qr = qlo[h] - qlo[h - 1]
                tabi = sh[m] * qr + (off[m] - qlo[h - 1])
            streams[h][c] = (w_e[m], slot[m], tabi)

    meta = []
    sl_all = []
    si_all = []
    for h in range(NP):
        E_h = max(len(streams[h][c][0]) for c in range(C))
        T_h = (E_h + P - 1) // P
        L = T_h * P
        wE = np.full((C, L), -1, np.int64)
        slE = np.zeros((C, L), np.int64)
        siE = np.zeros((C, L), np.int16)
        for c in range(C):
            w_e, slot, tabi = streams[h][c]
            n = len(w_e)
            wE[c, :n] = w_e
            slE[c, :n] = slot
            siE[c, :n] = tabi.astype(np.int16)
        wT = wE.reshape(C, T_h, P)
        wmask = np.ma.masked_equal(wT, -1)
        w0_t = np.asarray(wmask.min(axis=(0, 2)))       # [T]
        wmax_t = np.asarray(wmask.max(axis=(0, 2)))
        npl_t = wmax_t - w0_t + 1
        assert npl_t.max() <= 3, f"tile spans {npl_t.max()} windows"
        # window -> first/last tile with that window in span
        first_t = np.full(W, -1, np.int64)
        last_t = np.full(W, -1, np.int64)
        for t in range(T_h):
            for w in range(int(w0_t[t]), int(wmax_t[t]) + 1):
                if first_t[w] < 0:
                    first_t[w] = t
                last_t[w] = t
        assert (first_t >= 0).all()
        # two slot streams (bf16 integers are exact only up to 256):
        #   stream0: plane-0 edges -> slot (0..127), else PAD
        #   stream1: plane-1 edges -> slot, plane-2 -> 128+slot, else PAD
        plane = wE - w0_t.repeat(P)[None, :]
        v0 = np.where((wE >= 0) & (plane == 0), slE, PAD_SLOT)
        v1 = np.where((wE >= 0) & (plane == 1), slE,
                      np.where((wE >= 0) & (plane == 2), P + slE, PAD_SLOT))
        meta.append((T_h, w0_t.tolist(), npl_t.tolist(),
                     first_t.tolist(), last_t.tolist()))
        sl_all.append((v0.astype(np.float32), v1.astype(np.float32)))
        si_all.append(siE)

    per_core = []
    for c in range(C):
        si = np.concatenate([si_all[h][c] for h in range(NP)])
        sl0 = np.concatenate([sl_all[h][0][c] for h in range(NP)])
        sl1 = np.concatenate([sl_all[h][1][c] for h in range(NP)])
        T_tot = len(si) // P
        siw = np.ascontiguousarray(
            np.tile(si.reshape(T_tot * 8, 16).T, (8, 1)))
        sl = np.ascontiguousarray(np.concatenate(
            [sl0.reshape(T_tot, P).T, sl1.reshape(T_tot, P).T],
            axis=1).astype(ml_dtypes.bfloat16))
        per_core.append((siw, sl))
    return meta, per_core


# ------------------------------------------------------------- device build

def build_program(nc, N, H, F1, F2, meta, cc=True):
    """Emit the SPMD program. All cores run identical code; per-core data
    comes in through the input tensors."""
    NS = N // C
    W = (NS + P - 1) // P
    NSP = W * P
    NPASS = len(QW)      # pass 0: own-shard (local table); 1..: quarters
    NQ = len(QW) - 1
    qlo = [min(q * P, NS) for q in QW]   # quarter row bounds (local)
    qrows = [qlo[q + 1] - qlo[q] for q in range(NQ)]
    T_tot = sum(m[0] for m in meta)

    d_xT = nc.dram_tensor("xT", [H, NSP], BF16, kind="ExternalInput")
    d_W1 = nc.dram_tensor("W1", [H, F1], BF16, kind="ExternalInput")
    d_W2 = nc.dram_tensor("W2", [F1, F2], BF16, kind="ExternalInput")
    d_b1 = nc.dram_tensor("b1r", [P, F1], F32, kind="ExternalInput")
    d_b2 = nc.dram_tensor("b2r", [P, F2], F32, kind="ExternalInput")
    d_dinv = nc.dram_tensor("dinv", [P, W], F32, kind="ExternalInput")
    d_iota = nc.dram_tensor("iota", [P, 2 * P], BF16, kind="ExternalInput")
    d_ident = nc.dram_tensor("ident", [P, P], BF16, kind="ExternalInput")
    d_si = nc.dram_tensor("srcidx", [P, T_tot * 8], I16, kind="ExternalInput")
    d_sl = nc.dram_tensor("dstslot", [P, 2 * T_tot], BF16,
                          kind="ExternalInput")
    d_y = nc.dram_tensor("y", [NS, F2], F32, kind="ExternalOutput")

    with tile.TileContext(nc) as tc:
        with (
            tc.tile_pool(name="const", bufs=1) as const_pool,
            tc.tile_pool(name="persist", bufs=1) as persist,
            tc.tile_pool(name="gath", bufs=4) as gath_pool,
            tc.tile_pool(name="sbuild", bufs=3) as s_pool,
            tc.tile_pool(name="sdual", bufs=6) as sd_pool,
            tc.tile_pool(name="winbuf", bufs=3) as win_pool,
            tc.tile_pool(name="small", bufs=6) as small_pool,
            tc.tile_pool(name="agg", bufs=4, space="PSUM") as psum_agg,
            tc.tile_pool(name="dense", bufs=2, space="PSUM") as psum_dense,
            tc.tile_pool(name="tpose", bufs=2, space="PSUM") as psum_t,
            tc.tile_pool(name="dram", bufs=1, space="DRAM") as dram,
        ):
            # ---- constants / persistent state -----------------------------
            # dense-phase criticals first: the sync engine issues in order
            sb_xT = persist.tile([H, NSP], BF16, tag="xT")
            Q1 = QW[1] * P
            nc.sync.dma_start(out=sb_xT[:, :Q1], in_=d_xT[:, :Q1])
            sb_W1 = const_pool.tile([H, F1], BF16, tag="w1")
            nc.sync.dma_start(out=sb_W1[:], in_=d_W1[:])
            sb_dinv = const_pool.tile([P, W], F32, tag="dinv")
            nc.sync.dma_start(out=sb_dinv[:], in_=d_dinv[:])
            nc.sync.dma_start(out=sb_xT[:, Q1:], in_=d_xT[:, Q1:])
            sb_iota = const_pool.tile([P, 2, P], BF16, tag="iota")
            nc.scalar.dma_start(
                out=sb_iota[:], in_=d_iota[:].rearrange("p (k n) -> p k n", k=2))
            sb_W2 = const_pool.tile([F1, F2], BF16, tag="w2")
            nc.scalar.dma_start(out=sb_W2[:], in_=d_W2[:])
            sb_b1 = const_pool.tile([P, F1], F32, tag="b1")
            nc.scalar.dma_start(out=sb_b1[:], in_=d_b1[:])
            sb_b2 = const_pool.tile([P, F2], F32, tag="b2")
            nc.scalar.dma_start(out=sb_b2[:], in_=d_b2[:])
            sb_ident = const_pool.tile([P, P], BF16, tag="ident")
            nc.scalar.dma_start(out=sb_ident[:], in_=d_ident[:])
            # big edge tables last: they are not needed until the first
            # gather (~90us in) and would steal DMA bandwidth from xT
            sb_si = const_pool.tile([P, T_tot * 8], I16, tag="srcidx")
            nc.scalar.dma_start(out=sb_si[:], in_=d_si[:])
            sb_sl = const_pool.tile([P, 2 * T_tot], BF16, tag="dstslot")
            nc.scalar.dma_start(out=sb_sl[:], in_=d_sl[:])

            sb_hs1 = persist.tile([P, W, F1], BF16, tag="hs1")
            sb_hs2 = persist.tile([P, W, F2], F32, tag="hs2")
            sb_acc1 = persist.tile([P, W, F1], F32, tag="acc1")
            sb_acc2 = persist.tile([P, W, F2], F32, tag="acc2")

            # DRAM bounce + gather tables (own shard + per source quarter)
            hs1_own = dram.tile([NS, F1], BF16, name="hs1_own")
            hs2_own = dram.tile([NS, F2], F32, name="hs2_own")
            hs1_loc, hs1_full, hs2_loc, hs2_full = [], [], [], []
            for q in range(NQ):
                t1l = dram.tile([qrows[q], F1], BF16, name=f"hs1_loc{q}")
                t1f = dram.tile([C * qrows[q], F1], BF16,
                                name=f"hs1_full{q}", addr_space="Shared")
                t2l = dram.tile([qrows[q], F2], F32, name=f"hs2_loc{q}")
                t2f = dram.tile([C * qrows[q], F2], F32,
                                name=f"hs2_full{q}", addr_space="Shared")
                hs1_loc.append(t1l)
                hs1_full.append(t1f)
                hs2_loc.append(t2l)
                hs2_full.append(t2f)

            def allgather(src_t, dst_t):
                if cc:
                    nc.gpsimd.collective_compute(
                        "AllGather", mybir.AluOpType.bypass,
                        replica_groups=[list(range(C))],
                        ins=[src_t[:].opt()], outs=[dst_t[:].opt()])
                else:
                    rows = src_t.shape[0]
                    nc.sync.dma_start(out=dst_t[:rows, :], in_=src_t[:])



            def flush_quarter(loc_list, sb, q, F):
                """Batched write of one source-quarter of the window table:
                the full-window span as one partition-major DMA, plus the
                ragged last window (w = W-1) separately."""
                w0, w1 = QW[q], QW[q + 1]
                full_w = w1 - w0 if q < NQ - 1 else w1 - w0 - 1
                nc.scalar.dma_start(
                    out=loc_list[q][:full_w * P, :].rearrange(
                        "(w p) f -> p w f", p=P),
                    in_=sb[:, w0:w0 + full_w, :])
                if q == NQ - 1:
                    rows = NS - (W - 1) * P
                    nc.scalar.dma_start(
                        out=loc_list[q][full_w * P:, :],
                        in_=sb[:rows, W - 1, :])

            def flush_own(own_t, sb, q):
                """Local whole-shard table write (pass 0 gathers from it),
                one source-quarter chunk at a time."""
                w0, w1 = QW[q], QW[q + 1]
                full_w = w1 - w0 if q < NQ - 1 else w1 - w0 - 1
                nc.scalar.dma_start(
                    out=own_t[w0 * P:(w0 + full_w) * P, :].rearrange(
                        "(w p) f -> p w f", p=P),
                    in_=sb[:, w0:w0 + full_w, :])
                if q == NQ - 1:
                    rows = NS - (W - 1) * P
                    nc.scalar.dma_start(out=own_t[(W - 1) * P:, :],
                                        in_=sb[:rows, W - 1, :])

            # ---- phase 1: x0 = relu(x); hs1 = dinv * (x0 @ W1) ------------
            # psum region of 4 windows per bank; one batched dinv-mul per 4
            nc.vector.tensor_scalar_max(sb_xT[:, :Q1], sb_xT[:, :Q1], 0.0)
            nc.vector.tensor_scalar_max(sb_xT[:, Q1:], sb_xT[:, Q1:], 0.0)
            for q in range(NQ):
                for w0 in range(QW[q], QW[q + 1], 4):
                    n = min(4, QW[q + 1] - w0)
                    ph = psum_dense.tile([P, 4, F1], F32, tag="dense")
                    for k in range(n):
                        w = w0 + k
                        nc.tensor.matmul(ph[:, k, :],
                                         lhsT=sb_xT[:, w * P:(w + 1) * P],
                                         rhs=sb_W1[:], start=True, stop=True)
                    nc.vector.tensor_tensor(
                        out=sb_hs1[:, w0:w0 + n, :], in0=ph[:, :n, :],
                        in1=sb_dinv[:, w0:w0 + n].to_broadcast([P, n, F1]),
                        op=mybir.AluOpType.mult)
                flush_own(hs1_own, sb_hs1, q)
                flush_quarter(hs1_loc, sb_hs1, q, F1)
                allgather(hs1_loc[q], hs1_full[q])

            # ---- edge aggregation (both layers) ---------------------------
            def edge_layer(tables, F, dt, acc_sb, out_cb, ag_hook=None):
                """Two passes (one per src half); pass 0 parks the partial
                window sums in acc_sb, pass 1 finishes and calls out_cb.
                ag_hook(w) is called after window w is finalized in pass 1."""
                t_base = 0
                pa_w = {}
                for h in range(NPASS):
                    T_h, w0_t, npl_t, first_t, last_t = meta[h]
                    tab = tables[h]
                    # S batches built just-in-time inside the tile loop so the
                    # in-order vector queue interleaves builds with epilogues:
                    # plane-0 from stream0, plane-1 from stream1 (zero
                    # matrices for tiles without a plane-1 — harmless)
                    sts = {}
                    sts1 = {}
                    gts = {}
                    for t in range(T_h):
                        if t % TB == 0:
                            n = min(TB, T_h - t)
                            s = s_pool.tile([P, TB, P], dt, tag="sbuild")
                            nc.vector.tensor_tensor(
                                out=s[:, :n, :],
                                in0=sb_sl[:, t_base + t:t_base + t + n
                                          ].to_broadcast([P, n, P]),
                                in1=sb_iota[:, 0:1, :].to_broadcast([P, n, P]),
                                op=mybir.AluOpType.is_equal)
                            sts[t // TB] = s
                            if any(npl_t[tt] > 1 for tt in range(t, t + n)):
                                s1 = s_pool.tile([P, TB, P], dt, tag="sbuild1")
                                nc.vector.tensor_tensor(
                                    out=s1[:, :n, :],
                                    in0=sb_sl[:, T_tot + t_base + t:
                                              T_tot + t_base + t + n
                                              ].to_broadcast([P, n, P]),
                                    in1=sb_iota[:, 0:1, :].to_broadcast(
                                        [P, n, P]),
                                    op=mybir.AluOpType.is_equal)
                                sts1[t // TB] = s1
                        if t % TG == 0:
                            g = gath_pool.tile([P, TG, F], dt, tag="gath")
                            n = min(TG, T_h - t)
                            nc.gpsimd.dma_gather(
                                g[:, :n, :], tab[:],
                                sb_si[:, (t_base + t) * 8:(t_base + t + n) * 8],
                                n * P, n * P, F)
                            gts[t // TG] = g
                        rhs = gts[t // TG][:, t % TG, :]
                        for k in range(npl_t[t]):
                            w = w0_t[t] + k
                            if first_t[w] == t:
                                pa = psum_agg.tile([P, F1], F32, tag="agg",
                                                   name=f"pa{h}_{w}")
                                pa_w[w] = pa
                            if k == 0:
                                s_plane = sts[t // TB][:, t % TB, :]
                            elif k == 1:
                                s_plane = sts1[t // TB][:, t % TB, :]
                            else:
                                col = T_tot + t_base + t
                                sd = sd_pool.tile([P, 1, P], dt, tag="sd")
                                nc.vector.tensor_tensor(
                                    out=sd[:],
                                    in0=sb_sl[:, col:col + 1
                                              ].to_broadcast([P, 1, P]),
                                    in1=sb_iota[:, 1:2, :],
                                    op=mybir.AluOpType.is_equal)
                                s_plane = sd[:, 0, :]
                            nc.tensor.matmul(
                                pa_w[w][:, :F], lhsT=s_plane, rhs=rhs,
                                start=(first_t[w] == t),
                                stop=(last_t[w] == t))
                        for k in range(npl_t[t]):
                            w = w0_t[t] + k
                            if last_t[w] != t:
                                continue
                            pa = pa_w.pop(w)
                            rows = min(P, NS - w * P)
                            if h == 0:
                                nc.scalar.activation(
                                    acc_sb[:, w, :], pa[:, :F],
                                    mybir.ActivationFunctionType.Copy)
                            elif h < NPASS - 1:
                                nc.vector.tensor_tensor(
                                    out=acc_sb[:, w, :], in0=pa[:, :F],
                                    in1=acc_sb[:, w, :],
                                    op=mybir.AluOpType.add)
                            else:
                                out_cb(w, rows, pa[:, :F])
                                if ag_hook is not None:
                                    ag_hook(w)
                    t_base += T_h

            # ---- layer-1 epilogue: relu, transpose, dense L2 --------------
            def l1_out(w, rows, pa):
                tmp = win_pool.tile([P, F1], F32, tag="tmp")
                nc.vector.tensor_tensor(out=tmp[:], in0=pa,
                                        in1=sb_acc1[:, w, :],
                                        op=mybir.AluOpType.add)
                nc.vector.tensor_tensor(out=tmp[:], in0=tmp[:],
                                        in1=sb_hs1[:, w, :],
                                        op=mybir.AluOpType.add)
                nc.vector.tensor_scalar_mul(tmp[:], tmp[:], sb_dinv[:, w:w + 1])
                nc.vector.tensor_tensor(out=tmp[:], in0=tmp[:], in1=sb_b1[:],
                                        op=mybir.AluOpType.add)
                x1 = win_pool.tile([P, F1], BF16, tag="x1")
                nc.scalar.activation(x1[:], tmp[:],
                                     mybir.ActivationFunctionType.Relu)
                # transpose x1 -> lhsT for the layer-2 dense matmul
                pt = psum_t.tile([P, P], BF16, tag="tpose")
                nc.tensor.transpose(pt[:], x1[:], sb_ident[:])
                x1T = win_pool.tile([P, P], BF16, tag="x1T")
                nc.vector.tensor_copy(x1T[:], pt[:])
                ph2 = psum_dense.tile([P, F1], F32, tag="dense")
                nc.tensor.matmul(ph2[:, :F2], lhsT=x1T[:], rhs=sb_W2[:],
                                 start=True, stop=True)
                nc.vector.tensor_scalar_mul(
                    sb_hs2[:, w, :], ph2[:, :F2], sb_dinv[:, w:w + 1])

            qend = {QW[q + 1] - 1: q for q in range(NQ)}

            def l1_ag(w):
                if w in qend:
                    q = qend[w]
                    flush_own(hs2_own, sb_hs2, q)
                    flush_quarter(hs2_loc, sb_hs2, q, F2)
                    allgather(hs2_loc[q], hs2_full[q])

            edge_layer([hs1_own] + hs1_full, F1, BF16, sb_acc1, l1_out,
                       ag_hook=l1_ag)

            # ---- layer-2 edges + softmax ----------------------------------
            def l2_out(w, rows, pa):
                tmp = win_pool.tile([P, F2], F32, tag="tmp2")
                nc.vector.tensor_tensor(out=tmp[:], in0=pa,
                                        in1=sb_acc2[:, w, :],
                                        op=mybir.AluOpType.add)
                nc.vector.tensor_tensor(out=tmp[:], in0=tmp[:],
                                        in1=sb_hs2[:, w, :],
                                        op=mybir.AluOpType.add)
                nc.vector.tensor_scalar_mul(tmp[:], tmp[:], sb_dinv[:, w:w + 1])
                nc.vector.tensor_tensor(out=tmp[:], in0=tmp[:], in1=sb_b2[:],
                                        op=mybir.AluOpType.add)
                nmax = small_pool.tile([P, 1], F32, tag="nmax")
                nc.vector.tensor_reduce(nmax[:], tmp[:],
                                        axis=mybir.AxisListType.X,
                                        op=mybir.AluOpType.max, negate=True)
                ex = win_pool.tile([P, F2], F32, tag="ex")
                ssum = small_pool.tile([P, 1], F32, tag="ssum")
                nc.scalar.activation(ex[:], tmp[:],
                                     mybir.ActivationFunctionType.Exp,
                                     bias=nmax[:], accum_out=ssum[:])
                rsum = small_pool.tile([P, 1], F32, tag="rsum")
                nc.vector.reciprocal(rsum[:], ssum[:])
                yw = win_pool.tile([P, F2], F32, tag="yw")
                nc.vector.tensor_scalar_mul(yw[:], ex[:], rsum[:])
                nc.scalar.dma_start(out=d_y[w * P:w * P + rows, :],
                                    in_=yw[:rows, :])

            edge_layer([hs2_own] + hs2_full, F2, F32, sb_acc2, l2_out)

    return {"out_name": "y"}


# ---------------------------------------------------------------- frontend

_CACHE = {}


def _meta_key(meta):
    return tuple((T, tuple(w0), tuple(npl), tuple(ft), tuple(lt))
                 for T, w0, npl, ft, lt in meta)


def _build_and_compile(N, H, F1, F2, meta):
    nc = bacc.Bacc("TRN2", target_bir_lowering=False, debug=False,
                   enable_asserts=False, num_devices=C)
    build_program(nc, N, H, F1, F2, meta)
    nc.compile()
    return nc


def prepare_inputs(x, edge_index, W1, b1, W2, b2):
    N, H = x.shape
    F1 = W1.shape[1]
    F2 = W2.shape[1]
    NS = N // C
    W = (NS + P - 1) // P
    NSP = W * P

    src = np.asarray(edge_index[0], dtype=np.int64)
    dst = np.asarray(edge_index[1], dtype=np.int64)
    deg = np.bincount(dst, minlength=N).astype(np.float32) + 1.0
    dinv = (1.0 / np.sqrt(deg)).astype(np.float32)

    meta, per_core = _schedule(src, dst, N)

    iota = np.ascontiguousarray(np.tile(
        np.arange(2 * P, dtype=np.float32).reshape(1, -1),
        (P, 1)).astype(ml_dtypes.bfloat16))
    ident = np.eye(P, dtype=np.float32).astype(ml_dtypes.bfloat16)
    b1r = np.ascontiguousarray(np.tile(np.asarray(b1, np.float32), (P, 1)))
    b2r = np.ascontiguousarray(np.tile(np.asarray(b2, np.float32), (P, 1)))
    W1f = np.ascontiguousarray(
        np.asarray(W1, np.float32).astype(ml_dtypes.bfloat16))
    W2f = np.ascontiguousarray(
        np.asarray(W2, np.float32).astype(ml_dtypes.bfloat16))

    in_maps = []
    for c in range(C):
        xs = np.zeros((NSP, H), np.float32)
        xs[:NS] = np.asarray(x[c * NS:(c + 1) * NS], np.float32)
        xT = np.ascontiguousarray(xs.T.astype(ml_dtypes.bfloat16))
        dv = np.ones(NSP, np.float32)
        dv[:NS] = dinv[c * NS:(c + 1) * NS]
        dv = np.ascontiguousarray(dv.reshape(W, P).T)
        si, sl = per_core[c]
        in_maps.append({
            "xT": xT, "W1": W1f, "W2": W2f, "b1r": b1r, "b2r": b2r,
            "dinv": dv, "iota": iota, "ident": ident,
            "srcidx": si, "dstslot": sl,
        })
    return in_maps, (N, H, F1, F2, meta)


def kernel(x, edge_index, W1, b1, W2, b2, trace=False):
    x = np.asarray(x)
    in_maps, key = prepare_inputs(x, edge_index, W1, b1, W2, b2)
    N, H, F1, F2, meta = key
    ck = (N, H, F1, F2, _meta_key(meta))
    if ck not in _CACHE:
        _CACHE.clear()
        _CACHE[ck] = _build_and_compile(N, H, F1, F2, meta)
    nc = _CACHE[ck]
    res = run_bass_kernel_spmd(nc, in_maps, core_ids=list(range(C)),
                               trace=trace)
    y = np.concatenate([res.results[c]["y"] for c in range(C)], axis=0)
    if trace:
        kernel.last_exec_time_ns = res.exec_time_ns
    return y.astype(np.float32)


kernel.last_exec_time_ns = None
